# revision 20
# baseline (speedup 1.0000x reference)
"""Trainium2 Bass kernel for nn_Clip_OCR_Block (OCR attention block).

Sharding: 8 cores; core j handles image n=j//2, spatial half h=j%2
(8192 of 16384 pixels). The SpatialTemporalGather proxy needs a
full-image spatial reduction -> each core computes partial proxy
numerator/denominator over its half and pair-AllReduces with its
sibling core. Everything else is pixel-local.

All matmuls run as float32r (full PE speed at moving-dim >= 256,
~1.5e-4 relerr; moving dim must be a multiple of 4). BN scales are
folded into weights/biases on the host. Softmaxes skip
max-subtraction: |probs| <= ~5.5 and attention logits are in
[0.13, 0.58] for this problem's input distribution, so exp is safe.

Structure (one Tile graph; phases overlap via dependencies):
  A0: probs -> exp (+accum_out denominator) -> 64 PE transposes -> eT [s,k]
  B1 (16 tiles of 512 px): load F -> 16 PE transposes -> fT; proxy-num
      matmuls (accumulate [19,512] over 64 chunks); q1,q2 convs; q2 kept
      resident in SBUF.
  AllReduce proxy num+den with pair core; normalize; kk/val tiny convs.
  B2 (16 tiles): reload F; attention (logits/softmax/ctx); f_up; final
      conv on [ctx2 | F]; store.
"""
import numpy as np

import concourse.bacc as bacc
import concourse.mybir as mybir
import concourse.tile as tile
from concourse.bass_utils import run_bass_kernel_spmd

f32 = mybir.dt.float32
f32r = mybir.dt.float32r
AF = mybir.ActivationFunctionType

N, C, H, W = 4, 512, 128, 128
K, KC, OUT = 19, 256, 512
HW = H * W
HALF = HW // 2            # 8192 pixels per core
NCH = HALF // 128         # 64 chunks of 128 px
NT = HALF // 512          # 16 s-tiles of 512 px
SCALE = KC ** -0.5
KP = 20                   # K padded to a multiple of 4 (f32r moving-dim constraint)

_CACHED = {}


def _build_nc():
    nc = bacc.Bacc("TRN2", target_bir_lowering=False, debug=False, num_devices=8)

    feats_d = nc.dram_tensor("feats_half", [C, HALF], f32, kind="ExternalInput")
    probs_d = nc.dram_tensor("probs_half", [K, HALF], f32, kind="ExternalInput")
    wp1_d = nc.dram_tensor("wp1T", [C, KC], f32, kind="ExternalInput")
    wp2_d = nc.dram_tensor("wp2T", [KC, KC], f32, kind="ExternalInput")
    wo1_d = nc.dram_tensor("wo1T", [C, KC], f32, kind="ExternalInput")
    wo2_d = nc.dram_tensor("wo2T", [KC, KC], f32, kind="ExternalInput")
    wd_d = nc.dram_tensor("wdT", [C, KC], f32, kind="ExternalInput")
    wu_d = nc.dram_tensor("wuT", [KC, C], f32, kind="ExternalInput")
    wf_d = nc.dram_tensor("wfT", [2 * C, OUT], f32, kind="ExternalInput")
    bp1_d = nc.dram_tensor("bp1", [KC], f32, kind="ExternalInput")
    bp2_d = nc.dram_tensor("bp2", [KC], f32, kind="ExternalInput")
    bo1_d = nc.dram_tensor("bo1", [KC], f32, kind="ExternalInput")
    bo2_d = nc.dram_tensor("bo2", [KC], f32, kind="ExternalInput")
    bd_d = nc.dram_tensor("bd", [KC], f32, kind="ExternalInput")
    bu_d = nc.dram_tensor("bu", [C], f32, kind="ExternalInput")
    bf_d = nc.dram_tensor("bf", [OUT], f32, kind="ExternalInput")
    ident_d = nc.dram_tensor("ident", [128, 128], f32, kind="ExternalInput")
    ones_d = nc.dram_tensor("ones", [128, 32], f32, kind="ExternalInput")
    out_d = nc.dram_tensor("out_half", [OUT, HALF], f32, kind="ExternalOutput")

    prox_in = nc.dram_tensor("prox_in", [K, C + 1], f32)
    prox_out = nc.dram_tensor("prox_out", [K, C + 1], f32)
    q2_d = nc.dram_tensor("q2_spill", [NT, 2, 128, 512], f32)

    with tile.TileContext(nc) as tc:
        with nc.allow_low_precision(reason="f32r is fp32-width"), \
             tc.tile_pool(name="w", bufs=1) as wp, \
             tc.tile_pool(name="a", bufs=2) as ap_, \
             tc.tile_pool(name="b", bufs=2) as bp, \
             tc.tile_pool(name="psA", bufs=1, space="PSUM") as ppA, \
             tc.tile_pool(name="psT", bufs=2, space="PSUM") as ppT, \
             tc.tile_pool(name="psM", bufs=4, space="PSUM") as ppM:

            # ---- PE warmup: dummy matmuls on scratch while first DMAs land ----
            scratch = wp.tile([128, 512], f32, tag="scratch")
            nc.vector.memset(scratch[:], 0.0)
            for i in range(8):
                ps_w = ppM.tile([128, 512], f32, tag="mm", name="ps_warm")
                nc.tensor.matmul(ps_w[:], scratch[:, :128], scratch[:],
                                 start=True, stop=True)

            # ---- persistent weights / consts ----
            ident = wp.tile([128, 128], f32r, tag="ident")
            nc.sync.dma_start(ident[:], ident_d.ap().bitcast(f32r))
            ones = wp.tile([128, 32], f32r, tag="ones")
            nc.sync.dma_start(ones[:], ones_d.ap().bitcast(f32r))

            def wload(dram, kin, kout, tag):
                t = wp.tile([128, kin, kout, 128], f32r, tag=tag)
                nc.sync.dma_start(
                    t[:], dram.ap().rearrange(
                        "(k p) (o m) -> p k o m", p=128, m=128).bitcast(f32r))
                return t



            def bload(dram, nch, tag):
                t = wp.tile([128, nch], f32, tag=tag)
                nc.sync.dma_start(t[:], dram.ap().rearrange("(o p) -> p o", p=128))
                return t



            # ========== Phase A0: probs -> transpose -> exp -> eT [s,k] ==========
            # (transpose via f32r identity-matmul, exp fused into the PSUM->SBUF
            # copy on ACT; denominator = ones-matmul accumulation over eT chunks)
            eT = wp.tile([128, NCH, KP], f32r, tag="eT")
            ps_den = ppA.tile([KP, 32], f32, tag="den")
            PPS = 1024
            for pc in range(HALF // PPS):
                ppiece = ap_.tile([K, PPS], f32r, tag="ppiece")
                nc.sync.dma_start(ppiece[:],
                                  probs_d[:, pc * PPS:(pc + 1) * PPS].bitcast(f32r))
                ncc = PPS // 128
                ps_pe = ppT.tile([128, ncc, KP], f32, tag="tr", name="ps_pe")
                for c in range(ncc):
                    nc.tensor.matmul(ps_pe[:, c, :], ppiece[:, c * 128:(c + 1) * 128],
                                     ident[:K, :KP], start=True, stop=True)
                t0 = pc * ncc
                nc.scalar.activation(eT[:, t0:t0 + ncc, :], ps_pe[:], AF.Exp)
                for c in range(ncc):
                    t = t0 + c
                    nc.tensor.matmul(ps_den[:], eT[:, t, :], ones[:],
                                     start=(t == 0), stop=(t == NCH - 1))

            wp1 = wload(wp1_d, 4, 2, "wp1")
            wp2 = wload(wp2_d, 2, 2, "wp2")
            bp1 = bload(bp1_d, 2, "bp1")
            bp2 = bload(bp2_d, 2, "bp2")

            # ============ B1: F tiles -> fT transposes, proxy, q1, q2 ============
            ps_prox = ppA.tile([K, C], f32, tag="prox")
            deferred_q = []
            for t in range(NT):
                ft = bp.tile([128, 4, 512], f32r, tag="F1", bufs=3)
                nc.scalar.dma_start(
                    ft[:],
                    feats_d[:, t * 512:(t + 1) * 512].rearrange(
                        "(a p) s -> p a s", p=128).bitcast(f32r))

                for ss in range(4):
                    tt = t * 4 + ss
                    fT = ap_.tile([128, 512], f32r, tag="fT")
                    ps_t4 = ppT.tile([128, 4, 128], f32r, tag="tr", name="ps_t4")
                    for a in range(4):
                        nc.tensor.transpose(ps_t4[:, a, :],
                                            ft[:, a, ss * 128:(ss + 1) * 128],
                                            ident[:])
                    nc.vector.tensor_copy(fT[:], ps_t4[:].rearrange("s a c -> s (a c)"))
                    nc.tensor.matmul(ps_prox[:], eT[:, tt, 0:K], fT[:],
                                     start=(tt == 0), stop=(tt == NCH - 1))

                def qchain(t=t, ft=ft):
                    q1 = bp.tile([128, 2, 512], f32r, tag="q1", name="q1")
                    for o in range(2):
                        ps = ppM.tile([128, 512], f32, tag="mm", name="ps_q1")
                        for k in range(4):
                            nc.tensor.matmul(ps[:], wp1[:, k, o, :], ft[:, k, :],
                                             start=(k == 0), stop=(k == 3))
                        nc.scalar.activation(q1[:, o, :], ps[:], AF.Relu,
                                             bias=bp1[:, o:o + 1], scale=1.0)
                    q2 = bp.tile([128, 2, 512], f32r, tag="q2", name="q2")
                    for o in range(2):
                        ps = ppM.tile([128, 512], f32, tag="mm", name="ps_q2")
                        for k in range(2):
                            nc.tensor.matmul(ps[:], wp2[:, k, o, :], q1[:, k, :],
                                             start=(k == 0), stop=(k == 1))
                        nc.scalar.activation(q2[:, o, :], ps[:], AF.Relu,
                                             bias=bp2[:, o:o + 1], scale=1.0)
                    nc.sync.dma_start(
                        q2_d[t].rearrange("o p s -> p o s"), q2[:].bitcast(f32))
                if t >= NT - 2:
                    deferred_q.append(qchain)
                else:
                    qchain()

            # ---- B2 weights (loaded late so B1's F stream starts early) ----
            wo1 = wload(wo1_d, 4, 2, "wo1")
            wo2 = wload(wo2_d, 2, 2, "wo2")
            wd = wload(wd_d, 4, 2, "wd")
            wu = wload(wu_d, 2, 4, "wu")
            wf = wload(wf_d, 8, 4, "wf")
            bo1 = bload(bo1_d, 2, "bo1")
            bo2 = bload(bo2_d, 2, "bo2")
            bd = bload(bd_d, 2, "bd")
            bu = bload(bu_d, 4, "bu")
            bf = bload(bf_d, 4, "bf")

            # ============ AllReduce proxy partials with pair core ============
            prox_sb = wp.tile([K, C + 1], f32, tag="proxsb")
            nc.vector.tensor_copy(prox_sb[:, 1:], ps_prox[:])
            nc.vector.tensor_copy(prox_sb[:, 0:1], ps_den[:K, 0:1])
            nc.sync.dma_start(prox_in[:], prox_sb[:])
            nc.gpsimd.collective_compute(
                "AllReduce", mybir.AluOpType.add,
                replica_groups=[[0, 1], [2, 3], [4, 5], [6, 7]],
                ins=[prox_in[:]], outs=[prox_out[:]])
            for fn in deferred_q:
                fn()
            red = wp.tile([K, C + 1], f32, tag="red")
            nc.sync.dma_start(red[:], prox_out[:])
            recip = wp.tile([K, 1], f32, tag="recip")
            nc.vector.reciprocal(recip[:], red[:, 0:1])
            prox_n = wp.tile([K, C], f32r, tag="proxn")
            nc.vector.tensor_scalar_mul(prox_n[:], in0=red[:, 1:], scalar1=recip[:])

            # proxy -> [c, k] layout (identity matmul transposes)
            proxT = wp.tile([128, 4, KP], f32r, tag="proxT")
            for a in range(4):
                ps_t = ppT.tile([128, 128], f32, tag="tr", name="ps_tr2")
                nc.tensor.matmul(ps_t[:, :KP], prox_n[:, a * 128:(a + 1) * 128],
                                 ident[:K, :KP], start=True, stop=True)
                nc.vector.tensor_copy(proxT[:, a, :], ps_t[:, :KP])

            # kk = cbr(cbr(proxy, o1), o2); val = cbr(proxy, d)
            def small_conv(wt, bt, rhs_tile, kin, kout, tag):
                res = wp.tile([128, kout, KP], f32r, tag=tag)
                for o in range(kout):
                    ps = ppM.tile([128, 512], f32, tag="mm", name="ps_sc")
                    ps = ps[:, :KP]
                    for k in range(kin):
                        nc.tensor.matmul(ps[:], wt[:, k, o, :], rhs_tile[:, k, :],
                                         start=(k == 0), stop=(k == kin - 1))
                    nc.scalar.activation(res[:, o, :], ps[:], AF.Relu,
                                         bias=bt[:, o:o + 1], scale=1.0)
                return res

            kk1 = small_conv(wo1, bo1, proxT, 4, 2, "kk1")
            kk = small_conv(wo2, bo2, kk1, 2, 2, "kk")
            val_cb = small_conv(wd, bd, proxT, 4, 2, "valcb")
            valT = wp.tile([K, 2, 128], f32r, tag="valT")
            for o in range(2):
                ps_t = ppT.tile([128, 128], f32, tag="tr", name="ps_tr2")
                nc.tensor.matmul(ps_t[:K, :], val_cb[:, o, 0:K], ident[:],
                                 start=True, stop=True)
                nc.vector.tensor_copy(valT[:, o, :], ps_t[:K, :])

            # ============ B2: attention + f_up + final conv ============
            # Software-pipelined: tile t's attention chain (PE-light, full of
            # ACT/DVE latency) is interleaved with tile t-1's f_up/final conv
            # (PE-heavy) so the in-order PE stream never idles on the chain.
            st = [dict() for _ in range(NT)]

            def att1(t):
                d = st[t]
                ft = bp.tile([128, 4, 512], f32r, tag="F2", bufs=3, name="ft2")
                nc.scalar.dma_start(
                    ft[:],
                    feats_d[:, t * 512:(t + 1) * 512].rearrange(
                        "(a p) s -> p a s", p=128).bitcast(f32r))
                q2r = bp.tile([128, 2, 512], f32r, tag="q2r", bufs=3, name="q2r")
                nc.scalar.dma_start(q2r[:],
                                  q2_d[t].rearrange("o p s -> p o s").bitcast(f32r))
                ps_log = ppM.tile([128, 512], f32, tag="mm", name="ps_log")
                for k in range(2):
                    nc.tensor.matmul(ps_log[:K, :], kk[:, k, 0:K], q2r[:, k, :],
                                     start=(k == 0), stop=(k == 1))
                e_att = bp.tile([K, 512], f32r, tag="eatt", name="e_att")
                nc.scalar.activation(e_att[:], ps_log[:K, :], AF.Exp, scale=SCALE)
                d["ft"], d["e_att"] = ft, e_att

            def att2a(t):
                d = st[t]
                ps_dn = ppM.tile([128, 512], f32, tag="mm", name="ps_dn")
                nc.tensor.matmul(ps_dn[:1, :], ones[:K, 0:1], d["e_att"][:],
                                 start=True, stop=True)
                rc32 = bp.tile([1, 512], f32, tag="rc32", name="rc32")
                nc.vector.reciprocal_approx_fast(rc32[:], ps_dn[:1, :])
                rc = bp.tile([1, 512], f32r, tag="rc", name="rc")
                nc.vector.tensor_copy(rc[:], rc32[:])
                d["rc"] = rc

            def att2b(t):
                d = st[t]
                ps_bc = ppM.tile([128, 512], f32, tag="mm", name="ps_bc")
                nc.tensor.matmul(ps_bc[:K, :], ones[0:1, 0:K], d["rc"][:],
                                 start=True, stop=True)
                sim = bp.tile([K, 512], f32r, tag="sim", name="sim")
                nc.vector.tensor_mul(sim[:], d["e_att"][:].bitcast(f32), ps_bc[:K, :])
                d["sim"] = sim

            def att3(t):
                d = st[t]
                ctx = bp.tile([128, 2, 512], f32r, tag="ctx", name="ctx")
                for o in range(2):
                    ps = ppM.tile([128, 512], f32, tag="mm")
                    nc.tensor.matmul(ps[:], valT[:, o, :], d["sim"][:],
                                     start=True, stop=True)
                    nc.vector.tensor_copy(ctx[:, o, :], ps[:])
                d["ctx"] = ctx

            def fup(t, orange):
                d = st[t]
                if "ctx2" not in d:
                    d["ctx2"] = bp.tile([128, 4, 512], f32r, tag="ctx2", name="ctx2")
                for o in orange:
                    ps = ppM.tile([128, 512], f32, tag="mm")
                    for k in range(2):
                        nc.tensor.matmul(ps[:], wu[:, k, o, :], d["ctx"][:, k, :],
                                         start=(k == 0), stop=(k == 1))
                    nc.scalar.activation(d["ctx2"][:, o, :], ps[:], AF.Relu,
                                         bias=bu[:, o:o + 1], scale=1.0)

            def final(t, orange):
                d = st[t]
                if "ot" not in d:
                    d["ot"] = bp.tile([128, 4, 512], f32, tag="out", bufs=3,
                                      name="ot")
                ot = d["ot"]
                korder = [4, 5, 6, 7, 0, 1, 2, 3]
                for o in orange:
                    ps = ppM.tile([128, 512], f32, tag="mm")
                    for i, k in enumerate(korder):
                        rhs = d["ctx2"][:, k, :] if k < 4 else d["ft"][:, k - 4, :]
                        nc.tensor.matmul(ps[:], wf[:, k, o, :], rhs,
                                         start=(i == 0), stop=(i == 7))
                    nc.scalar.activation(ot[:, o, :], ps[:], AF.Relu,
                                         bias=bf[:, o:o + 1], scale=1.0)
                if orange[-1] == 3:
                    nc.sync.dma_start(
                        out_d[:, t * 512:(t + 1) * 512].rearrange(
                            "(o p) s -> p o s", p=128),
                        ot[:])
                    st[t] = None

            for t in range(NT + 1):
                if t < NT:
                    att1(t)
                if t >= 1:
                    fup(t - 1, (0, 1))
                if t < NT:
                    att2a(t)
                if t >= 1:
                    fup(t - 1, (2, 3))
                if t < NT:
                    att2b(t)
                if t >= 1:
                    final(t - 1, (0, 1))
                if t < NT:
                    att3(t)
                if t >= 1:
                    final(t - 1, (2, 3))

    nc.compile()
    return nc


def _fold(w, b, s, t):
    """conv+BN fold: y = s*(Wx+b)+t = (s.W)x + (s*b+t)."""
    w = np.asarray(w, np.float32)
    b = np.asarray(b, np.float32)
    s = np.asarray(s, np.float32)
    t = np.asarray(t, np.float32)
    return (s[:, None] * w), (s * b + t)


def kernel(feats, probs,
           wp1, bp1, sp1, tp1, wp2, bp2, sp2, tp2,
           wo1, bo1, so1, to1, wo2, bo2, so2, to2,
           wd, bd, sd, td, wu, bu, su, tu,
           wf, bf, sf, tf, clip_num, _trace=False):
    feats = np.ascontiguousarray(np.asarray(feats, np.float32))
    probs = np.ascontiguousarray(np.asarray(probs, np.float32))

    W1, B1 = _fold(wp1, bp1, sp1, tp1)
    W2, B2 = _fold(wp2, bp2, sp2, tp2)
    WO1, BO1 = _fold(wo1, bo1, so1, to1)
    WO2, BO2 = _fold(wo2, bo2, so2, to2)
    WD, BD = _fold(wd, bd, sd, td)
    WU, BU = _fold(wu, bu, su, tu)
    WF, BF = _fold(wf, bf, sf, tf)

    shared = {
        "wp1T": np.ascontiguousarray(W1.T), "bp1": B1,
        "wp2T": np.ascontiguousarray(W2.T), "bp2": B2,
        "wo1T": np.ascontiguousarray(WO1.T), "bo1": BO1,
        "wo2T": np.ascontiguousarray(WO2.T), "bo2": BO2,
        "wdT": np.ascontiguousarray(WD.T), "bd": BD,
        "wuT": np.ascontiguousarray(WU.T), "bu": BU,
        "wfT": np.ascontiguousarray(WF.T), "bf": BF,
        "ident": np.eye(128, dtype=np.float32),
        "ones": np.ones((128, 32), np.float32),
    }

    fr = feats.reshape(N, C, HW)
    pr = probs.reshape(N, K, HW)
    in_maps = []
    for j in range(8):
        n, h = j // 2, j % 2
        sl = slice(h * HALF, (h + 1) * HALF)
        in_maps.append({
            "feats_half": np.ascontiguousarray(fr[n, :, sl]),
            "probs_half": np.ascontiguousarray(pr[n, :, sl]),
            **shared,
        })

    if "nc" not in _CACHED:
        _CACHED["nc"] = _build_nc()
    nc = _CACHED["nc"]

    res = run_bass_kernel_spmd(nc, in_maps, list(range(8)), trace=_trace)
    out = np.empty((N, OUT, HW), np.float32)
    for j in range(8):
        n, h = j // 2, j % 2
        out[n, :, h * HALF:(h + 1) * HALF] = res.results[j]["out_half"]
    if _trace:
        kernel.last_exec_time_ns = res.exec_time_ns
        kernel.last_results = res
    return out.reshape(N, OUT, H, W)


# revision 21
# speedup vs baseline: 1.0381x; 1.0381x over previous
"""Trainium2 Bass kernel for nn_Clip_OCR_Block (OCR attention block).

Sharding: 8 cores; core j handles image n=j//2, spatial half h=j%2
(8192 of 16384 pixels). The SpatialTemporalGather proxy needs a
full-image spatial reduction -> each core computes partial proxy
numerator/denominator over its half and pair-AllReduces with its
sibling core. Everything else is pixel-local.

All matmuls run as float32r (full PE speed at moving-dim >= 256,
~1.5e-4 relerr; moving dim must be a multiple of 4). BN scales are
folded into weights/biases on the host. Softmaxes skip
max-subtraction: |probs| <= ~5.5 and attention logits are in
[0.13, 0.58] for this problem's input distribution, so exp is safe.

Structure (one Tile graph; phases overlap via dependencies):
  A0: probs -> exp (+accum_out denominator) -> 64 PE transposes -> eT [s,k]
  B1 (16 tiles of 512 px): load F -> 16 PE transposes -> fT; proxy-num
      matmuls (accumulate [19,512] over 64 chunks); q1,q2 convs; q2 kept
      resident in SBUF.
  AllReduce proxy num+den with pair core; normalize; kk/val tiny convs.
  B2 (16 tiles): reload F; attention (logits/softmax/ctx); f_up; final
      conv on [ctx2 | F]; store.
"""
import numpy as np

import concourse.bacc as bacc
import concourse.mybir as mybir
import concourse.tile as tile
from concourse.bass_utils import run_bass_kernel_spmd

f32 = mybir.dt.float32
f32r = mybir.dt.float32r
AF = mybir.ActivationFunctionType

N, C, H, W = 4, 512, 128, 128
K, KC, OUT = 19, 256, 512
HW = H * W
HALF = HW // 2            # 8192 pixels per core
NCH = HALF // 128         # 64 chunks of 128 px
NT = HALF // 512          # 16 s-tiles of 512 px
SCALE = KC ** -0.5
KP = 20                   # K padded to a multiple of 4 (f32r moving-dim constraint)

_CACHED = {}


def _build_nc():
    nc = bacc.Bacc("TRN2", target_bir_lowering=False, debug=False, num_devices=8)

    feats_d = nc.dram_tensor("feats_half", [C, HALF], f32, kind="ExternalInput")
    probs_d = nc.dram_tensor("probs_half", [K, HALF], f32, kind="ExternalInput")
    wp1_d = nc.dram_tensor("wp1T", [C, KC], f32, kind="ExternalInput")
    wp2_d = nc.dram_tensor("wp2T", [KC, KC], f32, kind="ExternalInput")
    wo1_d = nc.dram_tensor("wo1T", [C, KC], f32, kind="ExternalInput")
    wo2_d = nc.dram_tensor("wo2T", [KC, KC], f32, kind="ExternalInput")
    wd_d = nc.dram_tensor("wdT", [C, KC], f32, kind="ExternalInput")
    wu_d = nc.dram_tensor("wuT", [KC, C], f32, kind="ExternalInput")
    wf_d = nc.dram_tensor("wfT", [2 * C, OUT], f32, kind="ExternalInput")
    bp1_d = nc.dram_tensor("bp1", [KC], f32, kind="ExternalInput")
    bp2_d = nc.dram_tensor("bp2", [KC], f32, kind="ExternalInput")
    bo1_d = nc.dram_tensor("bo1", [KC], f32, kind="ExternalInput")
    bo2_d = nc.dram_tensor("bo2", [KC], f32, kind="ExternalInput")
    bd_d = nc.dram_tensor("bd", [KC], f32, kind="ExternalInput")
    bu_d = nc.dram_tensor("bu", [C], f32, kind="ExternalInput")
    bf_d = nc.dram_tensor("bf", [OUT], f32, kind="ExternalInput")
    ident_d = nc.dram_tensor("ident", [128, 128], f32, kind="ExternalInput")
    ones_d = nc.dram_tensor("ones", [128, 32], f32, kind="ExternalInput")
    out_d = nc.dram_tensor("out_half", [OUT, HALF], f32, kind="ExternalOutput")

    prox_in = nc.dram_tensor("prox_in", [K, C + 1], f32)
    prox_out = nc.dram_tensor("prox_out", [K, C + 1], f32)
    q2_d = nc.dram_tensor("q2_spill", [NT, 2, 128, 512], f32)

    with tile.TileContext(nc) as tc:
        with nc.allow_low_precision(reason="f32r is fp32-width"), \
             tc.tile_pool(name="w", bufs=1) as wp, \
             tc.tile_pool(name="a", bufs=2) as ap_, \
             tc.tile_pool(name="b", bufs=2) as bp, \
             tc.tile_pool(name="psA", bufs=1, space="PSUM") as ppA, \
             tc.tile_pool(name="psT", bufs=2, space="PSUM") as ppT, \
             tc.tile_pool(name="psM", bufs=4, space="PSUM") as ppM:

            # ---- PE warmup: dummy matmuls on scratch while first DMAs land ----
            scratch = wp.tile([128, 512], f32, tag="scratch")
            nc.vector.memset(scratch[:], 0.0)
            for i in range(8):
                ps_w = ppM.tile([128, 512], f32, tag="mm", name="ps_warm")
                nc.tensor.matmul(ps_w[:], scratch[:, :128], scratch[:],
                                 start=True, stop=True)

            # ---- persistent weights / consts ----
            ident = wp.tile([128, 128], f32r, tag="ident")
            nc.sync.dma_start(ident[:], ident_d.ap().bitcast(f32r))
            ones = wp.tile([128, 32], f32r, tag="ones")
            nc.sync.dma_start(ones[:], ones_d.ap().bitcast(f32r))

            def wload(dram, kin, kout, tag):
                t = wp.tile([128, kin, kout, 128], f32r, tag=tag)
                nc.sync.dma_start(
                    t[:], dram.ap().rearrange(
                        "(k p) (o m) -> p k o m", p=128, m=128).bitcast(f32r))
                return t



            def bload(dram, nch, tag):
                t = wp.tile([128, nch], f32, tag=tag)
                nc.sync.dma_start(t[:], dram.ap().rearrange("(o p) -> p o", p=128))
                return t



            # ========== Phase A0: probs -> transpose -> exp -> eT [s,k] ==========
            # (transpose via f32r identity-matmul, exp fused into the PSUM->SBUF
            # copy on ACT; denominator = ones-matmul accumulation over eT chunks)
            eT = wp.tile([128, NCH, KP], f32r, tag="eT")
            ps_den = ppA.tile([KP, 32], f32, tag="den")
            PPS = 1024
            for pc in range(HALF // PPS):
                ppiece = ap_.tile([K, PPS], f32r, tag="ppiece")
                nc.sync.dma_start(ppiece[:],
                                  probs_d[:, pc * PPS:(pc + 1) * PPS].bitcast(f32r))
                ncc = PPS // 128
                ps_pe = ppT.tile([128, ncc, KP], f32, tag="tr", name="ps_pe")
                for c in range(ncc):
                    nc.tensor.matmul(ps_pe[:, c, :], ppiece[:, c * 128:(c + 1) * 128],
                                     ident[:K, :KP], start=True, stop=True)
                t0 = pc * ncc
                nc.scalar.activation(eT[:, t0:t0 + ncc, :], ps_pe[:], AF.Exp)
                for c in range(ncc):
                    t = t0 + c
                    nc.tensor.matmul(ps_den[:], eT[:, t, :], ones[:],
                                     start=(t == 0), stop=(t == NCH - 1))

            wp1 = wload(wp1_d, 4, 2, "wp1")
            wp2 = wload(wp2_d, 2, 2, "wp2")
            bp1 = bload(bp1_d, 2, "bp1")
            bp2 = bload(bp2_d, 2, "bp2")

            # ============ B1: F tiles -> fT transposes, proxy, q1, q2 ============
            ps_prox = ppA.tile([K, C], f32, tag="prox")
            deferred_q = []
            for t in range(NT):
                ft = bp.tile([128, 4, 512], f32r, tag="F1", bufs=3)
                nc.sync.dma_start(
                    ft[:],
                    feats_d[:, t * 512:(t + 1) * 512].rearrange(
                        "(a p) s -> p a s", p=128).bitcast(f32r))

                for ss in range(4):
                    tt = t * 4 + ss
                    fT = ap_.tile([128, 512], f32r, tag="fT")
                    ps_t4 = ppT.tile([128, 4, 128], f32r, tag="tr", name="ps_t4")
                    for a in range(4):
                        nc.tensor.transpose(ps_t4[:, a, :],
                                            ft[:, a, ss * 128:(ss + 1) * 128],
                                            ident[:])
                    nc.vector.tensor_copy(fT[:], ps_t4[:].rearrange("s a c -> s (a c)"))
                    nc.tensor.matmul(ps_prox[:], eT[:, tt, 0:K], fT[:],
                                     start=(tt == 0), stop=(tt == NCH - 1))

                def qchain(t=t, ft=ft):
                    q1 = bp.tile([128, 2, 512], f32r, tag="q1", name="q1")
                    for o in range(2):
                        ps = ppM.tile([128, 512], f32, tag="mm", name="ps_q1")
                        for k in range(4):
                            nc.tensor.matmul(ps[:], wp1[:, k, o, :], ft[:, k, :],
                                             start=(k == 0), stop=(k == 3))
                        nc.scalar.activation(q1[:, o, :], ps[:], AF.Relu,
                                             bias=bp1[:, o:o + 1], scale=1.0)
                    q2 = bp.tile([128, 2, 512], f32r, tag="q2", name="q2")
                    for o in range(2):
                        ps = ppM.tile([128, 512], f32, tag="mm", name="ps_q2")
                        for k in range(2):
                            nc.tensor.matmul(ps[:], wp2[:, k, o, :], q1[:, k, :],
                                             start=(k == 0), stop=(k == 1))
                        nc.scalar.activation(q2[:, o, :], ps[:], AF.Relu,
                                             bias=bp2[:, o:o + 1], scale=1.0)
                    nc.sync.dma_start(
                        q2_d[t].rearrange("o p s -> p o s"), q2[:].bitcast(f32))
                if t >= NT - 2:
                    deferred_q.append(qchain)
                else:
                    qchain()

            # ---- B2 weights (loaded late so B1's F stream starts early) ----
            wo1 = wload(wo1_d, 4, 2, "wo1")
            wo2 = wload(wo2_d, 2, 2, "wo2")
            wd = wload(wd_d, 4, 2, "wd")
            wu = wload(wu_d, 2, 4, "wu")
            wf = wload(wf_d, 8, 4, "wf")
            bo1 = bload(bo1_d, 2, "bo1")
            bo2 = bload(bo2_d, 2, "bo2")
            bd = bload(bd_d, 2, "bd")
            bu = bload(bu_d, 4, "bu")
            bf = bload(bf_d, 4, "bf")

            # ============ AllReduce proxy partials with pair core ============
            prox_sb = wp.tile([K, C + 1], f32, tag="proxsb")
            nc.vector.tensor_copy(prox_sb[:, 1:], ps_prox[:])
            nc.vector.tensor_copy(prox_sb[:, 0:1], ps_den[:K, 0:1])
            nc.sync.dma_start(prox_in[:], prox_sb[:])
            nc.gpsimd.collective_compute(
                "AllReduce", mybir.AluOpType.add,
                replica_groups=[[0, 1], [2, 3], [4, 5], [6, 7]],
                ins=[prox_in[:]], outs=[prox_out[:]])
            for fn in deferred_q:
                fn()
            red = wp.tile([K, C + 1], f32, tag="red")
            nc.sync.dma_start(red[:], prox_out[:])
            recip = wp.tile([K, 1], f32, tag="recip")
            nc.vector.reciprocal(recip[:], red[:, 0:1])
            prox_n = wp.tile([K, C], f32r, tag="proxn")
            nc.vector.tensor_scalar_mul(prox_n[:], in0=red[:, 1:], scalar1=recip[:])

            # proxy -> [c, k] layout (identity matmul transposes)
            proxT = wp.tile([128, 4, KP], f32r, tag="proxT")
            for a in range(4):
                ps_t = ppT.tile([128, 128], f32, tag="tr", name="ps_tr2")
                nc.tensor.matmul(ps_t[:, :KP], prox_n[:, a * 128:(a + 1) * 128],
                                 ident[:K, :KP], start=True, stop=True)
                nc.vector.tensor_copy(proxT[:, a, :], ps_t[:, :KP])

            # kk = cbr(cbr(proxy, o1), o2); val = cbr(proxy, d)
            def small_conv(wt, bt, rhs_tile, kin, kout, tag):
                res = wp.tile([128, kout, KP], f32r, tag=tag)
                for o in range(kout):
                    ps = ppM.tile([128, 512], f32, tag="mm", name="ps_sc")
                    ps = ps[:, :KP]
                    for k in range(kin):
                        nc.tensor.matmul(ps[:], wt[:, k, o, :], rhs_tile[:, k, :],
                                         start=(k == 0), stop=(k == kin - 1))
                    nc.scalar.activation(res[:, o, :], ps[:], AF.Relu,
                                         bias=bt[:, o:o + 1], scale=1.0)
                return res

            kk1 = small_conv(wo1, bo1, proxT, 4, 2, "kk1")
            kk = small_conv(wo2, bo2, kk1, 2, 2, "kk")
            val_cb = small_conv(wd, bd, proxT, 4, 2, "valcb")
            valT = wp.tile([K, 2, 128], f32r, tag="valT")
            for o in range(2):
                ps_t = ppT.tile([128, 128], f32, tag="tr", name="ps_tr2")
                nc.tensor.matmul(ps_t[:K, :], val_cb[:, o, 0:K], ident[:],
                                 start=True, stop=True)
                nc.vector.tensor_copy(valT[:, o, :], ps_t[:K, :])

            # ============ B2: attention + f_up + final conv ============
            # Software-pipelined: tile t's attention chain (PE-light, full of
            # ACT/DVE latency) is interleaved with tile t-1's f_up/final conv
            # (PE-heavy) so the in-order PE stream never idles on the chain.
            st = [dict() for _ in range(NT)]

            def att1(t):
                d = st[t]
                ft = bp.tile([128, 4, 512], f32r, tag="F2", bufs=3, name="ft2")
                nc.sync.dma_start(
                    ft[:],
                    feats_d[:, t * 512:(t + 1) * 512].rearrange(
                        "(a p) s -> p a s", p=128).bitcast(f32r))
                q2r = bp.tile([128, 2, 512], f32r, tag="q2r", bufs=3, name="q2r")
                nc.sync.dma_start(q2r[:],
                                  q2_d[t].rearrange("o p s -> p o s").bitcast(f32r))
                ps_log = ppM.tile([128, 512], f32, tag="mm", name="ps_log")
                for k in range(2):
                    nc.tensor.matmul(ps_log[:K, :], kk[:, k, 0:K], q2r[:, k, :],
                                     start=(k == 0), stop=(k == 1))
                e_att = bp.tile([K, 512], f32r, tag="eatt", name="e_att")
                nc.scalar.activation(e_att[:], ps_log[:K, :], AF.Exp, scale=SCALE)
                d["ft"], d["e_att"] = ft, e_att

            def att2a(t):
                d = st[t]
                ps_dn = ppM.tile([128, 512], f32, tag="mm", name="ps_dn")
                nc.tensor.matmul(ps_dn[:1, :], ones[:K, 0:1], d["e_att"][:],
                                 start=True, stop=True)
                rc32 = bp.tile([1, 512], f32, tag="rc32", name="rc32")
                nc.vector.reciprocal_approx_fast(rc32[:], ps_dn[:1, :])
                rc = bp.tile([1, 512], f32r, tag="rc", name="rc")
                nc.vector.tensor_copy(rc[:], rc32[:])
                d["rc"] = rc

            def att2b(t):
                d = st[t]
                ps_bc = ppM.tile([128, 512], f32, tag="mm", name="ps_bc")
                nc.tensor.matmul(ps_bc[:K, :], ones[0:1, 0:K], d["rc"][:],
                                 start=True, stop=True)
                sim = bp.tile([K, 512], f32r, tag="sim", name="sim")
                nc.vector.tensor_mul(sim[:], d["e_att"][:].bitcast(f32), ps_bc[:K, :])
                d["sim"] = sim

            def att3(t):
                d = st[t]
                ctx = bp.tile([128, 2, 512], f32r, tag="ctx", name="ctx")
                for o in range(2):
                    ps = ppM.tile([128, 512], f32, tag="mm")
                    nc.tensor.matmul(ps[:], valT[:, o, :], d["sim"][:],
                                     start=True, stop=True)
                    nc.vector.tensor_copy(ctx[:, o, :], ps[:])
                d["ctx"] = ctx

            def fup(t, orange):
                d = st[t]
                if "ctx2" not in d:
                    d["ctx2"] = bp.tile([128, 4, 512], f32r, tag="ctx2", name="ctx2")
                for o in orange:
                    ps = ppM.tile([128, 512], f32, tag="mm")
                    for k in range(2):
                        nc.tensor.matmul(ps[:], wu[:, k, o, :], d["ctx"][:, k, :],
                                         start=(k == 0), stop=(k == 1))
                    nc.scalar.activation(d["ctx2"][:, o, :], ps[:], AF.Relu,
                                         bias=bu[:, o:o + 1], scale=1.0)

            def final(t, orange):
                d = st[t]
                if "ot" not in d:
                    d["ot"] = bp.tile([128, 4, 512], f32, tag="out", bufs=3,
                                      name="ot")
                ot = d["ot"]
                korder = [4, 5, 6, 7, 0, 1, 2, 3]
                for o in orange:
                    ps = ppM.tile([128, 512], f32, tag="mm")
                    for i, k in enumerate(korder):
                        rhs = d["ctx2"][:, k, :] if k < 4 else d["ft"][:, k - 4, :]
                        nc.tensor.matmul(ps[:], wf[:, k, o, :], rhs,
                                         start=(i == 0), stop=(i == 7))
                    nc.scalar.activation(ot[:, o, :], ps[:], AF.Relu,
                                         bias=bf[:, o:o + 1], scale=1.0)
                if orange[-1] == 3:
                    nc.sync.dma_start(
                        out_d[:, t * 512:(t + 1) * 512].rearrange(
                            "(o p) s -> p o s", p=128),
                        ot[:])
                    st[t] = None

            for t in range(NT + 1):
                if t < NT:
                    att1(t)
                if t >= 1:
                    fup(t - 1, (0, 1))
                if t < NT:
                    att2a(t)
                if t >= 1:
                    fup(t - 1, (2, 3))
                if t < NT:
                    att2b(t)
                if t >= 1:
                    final(t - 1, (0, 1))
                if t < NT:
                    att3(t)
                if t >= 1:
                    final(t - 1, (2, 3))

    nc.compile()
    return nc


def _fold(w, b, s, t):
    """conv+BN fold: y = s*(Wx+b)+t = (s.W)x + (s*b+t)."""
    w = np.asarray(w, np.float32)
    b = np.asarray(b, np.float32)
    s = np.asarray(s, np.float32)
    t = np.asarray(t, np.float32)
    return (s[:, None] * w), (s * b + t)


def kernel(feats, probs,
           wp1, bp1, sp1, tp1, wp2, bp2, sp2, tp2,
           wo1, bo1, so1, to1, wo2, bo2, so2, to2,
           wd, bd, sd, td, wu, bu, su, tu,
           wf, bf, sf, tf, clip_num, _trace=False):
    feats = np.ascontiguousarray(np.asarray(feats, np.float32))
    probs = np.ascontiguousarray(np.asarray(probs, np.float32))

    W1, B1 = _fold(wp1, bp1, sp1, tp1)
    W2, B2 = _fold(wp2, bp2, sp2, tp2)
    WO1, BO1 = _fold(wo1, bo1, so1, to1)
    WO2, BO2 = _fold(wo2, bo2, so2, to2)
    WD, BD = _fold(wd, bd, sd, td)
    WU, BU = _fold(wu, bu, su, tu)
    WF, BF = _fold(wf, bf, sf, tf)

    shared = {
        "wp1T": np.ascontiguousarray(W1.T), "bp1": B1,
        "wp2T": np.ascontiguousarray(W2.T), "bp2": B2,
        "wo1T": np.ascontiguousarray(WO1.T), "bo1": BO1,
        "wo2T": np.ascontiguousarray(WO2.T), "bo2": BO2,
        "wdT": np.ascontiguousarray(WD.T), "bd": BD,
        "wuT": np.ascontiguousarray(WU.T), "bu": BU,
        "wfT": np.ascontiguousarray(WF.T), "bf": BF,
        "ident": np.eye(128, dtype=np.float32),
        "ones": np.ones((128, 32), np.float32),
    }

    fr = feats.reshape(N, C, HW)
    pr = probs.reshape(N, K, HW)
    in_maps = []
    for j in range(8):
        n, h = j // 2, j % 2
        sl = slice(h * HALF, (h + 1) * HALF)
        in_maps.append({
            "feats_half": np.ascontiguousarray(fr[n, :, sl]),
            "probs_half": np.ascontiguousarray(pr[n, :, sl]),
            **shared,
        })

    if "nc" not in _CACHED:
        _CACHED["nc"] = _build_nc()
    nc = _CACHED["nc"]

    res = run_bass_kernel_spmd(nc, in_maps, list(range(8)), trace=_trace)
    out = np.empty((N, OUT, HW), np.float32)
    for j in range(8):
        n, h = j // 2, j % 2
        out[n, :, h * HALF:(h + 1) * HALF] = res.results[j]["out_half"]
    if _trace:
        kernel.last_exec_time_ns = res.exec_time_ns
        kernel.last_results = res
    return out.reshape(N, OUT, H, W)


# revision 22
# speedup vs baseline: 1.0443x; 1.0060x over previous
"""Trainium2 Bass kernel for nn_Clip_OCR_Block (OCR attention block).

Sharding: 8 cores; core j handles image n=j//2, spatial half h=j%2
(8192 of 16384 pixels). The SpatialTemporalGather proxy needs a
full-image spatial reduction -> each core computes partial proxy
numerator/denominator over its half and pair-AllReduces with its
sibling core. Everything else is pixel-local.

All matmuls run as float32r (full PE speed at moving-dim >= 256,
~1.5e-4 relerr; moving dim must be a multiple of 4). BN scales are
folded into weights/biases on the host. Softmaxes skip
max-subtraction: |probs| <= ~5.5 and attention logits are in
[0.13, 0.58] for this problem's input distribution, so exp is safe.

Structure (one Tile graph; phases overlap via dependencies):
  A0: probs -> exp (+accum_out denominator) -> 64 PE transposes -> eT [s,k]
  B1 (16 tiles of 512 px): load F -> 16 PE transposes -> fT; proxy-num
      matmuls (accumulate [19,512] over 64 chunks); q1,q2 convs; q2 kept
      resident in SBUF.
  AllReduce proxy num+den with pair core; normalize; kk/val tiny convs.
  B2 (16 tiles): reload F; attention (logits/softmax/ctx); f_up; final
      conv on [ctx2 | F]; store.
"""
import numpy as np

import concourse.bacc as bacc
import concourse.mybir as mybir
import concourse.tile as tile
from concourse.bass_utils import run_bass_kernel_spmd

f32 = mybir.dt.float32
f32r = mybir.dt.float32r
AF = mybir.ActivationFunctionType

N, C, H, W = 4, 512, 128, 128
K, KC, OUT = 19, 256, 512
HW = H * W
HALF = HW // 2            # 8192 pixels per core
NCH = HALF // 128         # 64 chunks of 128 px
NT = HALF // 512          # 16 s-tiles of 512 px
SCALE = KC ** -0.5
KP = 20                   # K padded to a multiple of 4 (f32r moving-dim constraint)

_CACHED = {}


def _build_nc():
    nc = bacc.Bacc("TRN2", target_bir_lowering=False, debug=False, num_devices=8)

    feats_d = nc.dram_tensor("feats_half", [C, HALF], f32, kind="ExternalInput")
    probs_d = nc.dram_tensor("probs_half", [K, HALF], f32, kind="ExternalInput")
    wp1_d = nc.dram_tensor("wp1T", [C, KC], f32, kind="ExternalInput")
    wp2_d = nc.dram_tensor("wp2T", [KC, KC], f32, kind="ExternalInput")
    wo1_d = nc.dram_tensor("wo1T", [C, KC], f32, kind="ExternalInput")
    wo2_d = nc.dram_tensor("wo2T", [KC, KC], f32, kind="ExternalInput")
    wd_d = nc.dram_tensor("wdT", [C, KC], f32, kind="ExternalInput")
    wu_d = nc.dram_tensor("wuT", [KC, C], f32, kind="ExternalInput")
    wf_d = nc.dram_tensor("wfT", [2 * C, OUT], f32, kind="ExternalInput")
    bp1_d = nc.dram_tensor("bp1", [KC], f32, kind="ExternalInput")
    bp2_d = nc.dram_tensor("bp2", [KC], f32, kind="ExternalInput")
    bo1_d = nc.dram_tensor("bo1", [KC], f32, kind="ExternalInput")
    bo2_d = nc.dram_tensor("bo2", [KC], f32, kind="ExternalInput")
    bd_d = nc.dram_tensor("bd", [KC], f32, kind="ExternalInput")
    bu_d = nc.dram_tensor("bu", [C], f32, kind="ExternalInput")
    bf_d = nc.dram_tensor("bf", [OUT], f32, kind="ExternalInput")
    ident_d = nc.dram_tensor("ident", [128, 128], f32, kind="ExternalInput")
    ones_d = nc.dram_tensor("ones", [128, 32], f32, kind="ExternalInput")
    out_d = nc.dram_tensor("out_half", [OUT, HALF], f32, kind="ExternalOutput")

    prox_in = nc.dram_tensor("prox_in", [K, C + 1], f32)
    prox_out = nc.dram_tensor("prox_out", [K, C + 1], f32)
    q2_d = nc.dram_tensor("q2_spill", [NT, 2, 128, 512], f32)

    with tile.TileContext(nc) as tc:
        with nc.allow_low_precision(reason="f32r is fp32-width"), \
             tc.tile_pool(name="w", bufs=1) as wp, \
             tc.tile_pool(name="a", bufs=2) as ap_, \
             tc.tile_pool(name="b", bufs=2) as bp, \
             tc.tile_pool(name="psA", bufs=1, space="PSUM") as ppA, \
             tc.tile_pool(name="psT", bufs=2, space="PSUM") as ppT, \
             tc.tile_pool(name="psM", bufs=4, space="PSUM") as ppM:

            # ---- PE warmup: dummy matmuls on scratch while first DMAs land ----
            scratch = wp.tile([128, 512], f32, tag="scratch")
            nc.vector.memset(scratch[:], 0.0)
            for i in range(16):
                ps_w = ppM.tile([128, 512], f32, tag="mm", name="ps_warm")
                nc.tensor.matmul(ps_w[:], scratch[:, :128], scratch[:],
                                 start=True, stop=True)

            # ---- persistent weights / consts ----
            ident = wp.tile([128, 128], f32r, tag="ident")
            nc.sync.dma_start(ident[:], ident_d.ap().bitcast(f32r))
            ones = wp.tile([128, 32], f32r, tag="ones")
            nc.sync.dma_start(ones[:], ones_d.ap().bitcast(f32r))

            def wload(dram, kin, kout, tag):
                t = wp.tile([128, kin, kout, 128], f32r, tag=tag)
                nc.sync.dma_start(
                    t[:], dram.ap().rearrange(
                        "(k p) (o m) -> p k o m", p=128, m=128).bitcast(f32r))
                return t



            def bload(dram, nch, tag):
                t = wp.tile([128, nch], f32, tag=tag)
                nc.sync.dma_start(t[:], dram.ap().rearrange("(o p) -> p o", p=128))
                return t



            # ========== Phase A0: probs -> transpose -> exp -> eT [s,k] ==========
            # (transpose via f32r identity-matmul, exp fused into the PSUM->SBUF
            # copy on ACT; denominator = ones-matmul accumulation over eT chunks)
            eT = wp.tile([128, NCH, KP], f32r, tag="eT")
            ps_den = ppA.tile([KP, 32], f32, tag="den")
            PPS = 1024
            for pc in range(HALF // PPS):
                ppiece = ap_.tile([K, PPS], f32r, tag="ppiece")
                nc.sync.dma_start(ppiece[:],
                                  probs_d[:, pc * PPS:(pc + 1) * PPS].bitcast(f32r))
                ncc = PPS // 128
                ps_pe = ppT.tile([128, ncc, KP], f32, tag="tr", name="ps_pe")
                for c in range(ncc):
                    nc.tensor.matmul(ps_pe[:, c, :], ppiece[:, c * 128:(c + 1) * 128],
                                     ident[:K, :KP], start=True, stop=True)
                t0 = pc * ncc
                nc.scalar.activation(eT[:, t0:t0 + ncc, :], ps_pe[:], AF.Exp)
                for c in range(ncc):
                    t = t0 + c
                    nc.tensor.matmul(ps_den[:], eT[:, t, :], ones[:],
                                     start=(t == 0), stop=(t == NCH - 1))

            wp1 = wload(wp1_d, 4, 2, "wp1")
            wp2 = wload(wp2_d, 2, 2, "wp2")
            bp1 = bload(bp1_d, 2, "bp1")
            bp2 = bload(bp2_d, 2, "bp2")

            # ============ B1: F tiles -> fT transposes, proxy, q1, q2 ============
            ps_prox = ppA.tile([K, C], f32, tag="prox")
            deferred_q = []
            for t in range(NT):
                ft = bp.tile([128, 4, 512], f32r, tag="F1", bufs=3)
                nc.sync.dma_start(
                    ft[:],
                    feats_d[:, t * 512:(t + 1) * 512].rearrange(
                        "(a p) s -> p a s", p=128).bitcast(f32r))

                for ss in range(4):
                    tt = t * 4 + ss
                    fT = ap_.tile([128, 512], f32r, tag="fT")
                    ps_t4 = ppT.tile([128, 4, 128], f32r, tag="tr", name="ps_t4")
                    for a in range(4):
                        nc.tensor.transpose(ps_t4[:, a, :],
                                            ft[:, a, ss * 128:(ss + 1) * 128],
                                            ident[:])
                    nc.vector.tensor_copy(fT[:], ps_t4[:].rearrange("s a c -> s (a c)"))
                    nc.tensor.matmul(ps_prox[:], eT[:, tt, 0:K], fT[:],
                                     start=(tt == 0), stop=(tt == NCH - 1))

                def qchain(t=t, ft=ft):
                    q1 = bp.tile([128, 2, 512], f32r, tag="q1", name="q1")
                    for o in range(2):
                        ps = ppM.tile([128, 512], f32, tag="mm", name="ps_q1")
                        for k in range(4):
                            nc.tensor.matmul(ps[:], wp1[:, k, o, :], ft[:, k, :],
                                             start=(k == 0), stop=(k == 3))
                        nc.scalar.activation(q1[:, o, :], ps[:], AF.Relu,
                                             bias=bp1[:, o:o + 1], scale=1.0)
                    q2 = bp.tile([128, 2, 512], f32r, tag="q2", name="q2")
                    for o in range(2):
                        ps = ppM.tile([128, 512], f32, tag="mm", name="ps_q2")
                        for k in range(2):
                            nc.tensor.matmul(ps[:], wp2[:, k, o, :], q1[:, k, :],
                                             start=(k == 0), stop=(k == 1))
                        nc.scalar.activation(q2[:, o, :], ps[:], AF.Relu,
                                             bias=bp2[:, o:o + 1], scale=1.0)
                    nc.sync.dma_start(
                        q2_d[t].rearrange("o p s -> p o s"), q2[:].bitcast(f32))
                if t >= NT - 2:
                    deferred_q.append(qchain)
                else:
                    qchain()

            # ---- B2 weights (loaded late so B1's F stream starts early) ----
            wo1 = wload(wo1_d, 4, 2, "wo1")
            wo2 = wload(wo2_d, 2, 2, "wo2")
            wd = wload(wd_d, 4, 2, "wd")
            wu = wload(wu_d, 2, 4, "wu")
            wf = wload(wf_d, 8, 4, "wf")
            bo1 = bload(bo1_d, 2, "bo1")
            bo2 = bload(bo2_d, 2, "bo2")
            bd = bload(bd_d, 2, "bd")
            bu = bload(bu_d, 4, "bu")
            bf = bload(bf_d, 4, "bf")

            # ============ AllReduce proxy partials with pair core ============
            prox_sb = wp.tile([K, C + 1], f32, tag="proxsb")
            nc.vector.tensor_copy(prox_sb[:, 1:], ps_prox[:])
            nc.vector.tensor_copy(prox_sb[:, 0:1], ps_den[:K, 0:1])
            nc.sync.dma_start(prox_in[:], prox_sb[:])
            nc.gpsimd.collective_compute(
                "AllReduce", mybir.AluOpType.add,
                replica_groups=[[0, 1], [2, 3], [4, 5], [6, 7]],
                ins=[prox_in[:]], outs=[prox_out[:]])
            for fn in deferred_q:
                fn()
            red = wp.tile([K, C + 1], f32, tag="red")
            nc.sync.dma_start(red[:], prox_out[:])
            recip = wp.tile([K, 1], f32, tag="recip")
            nc.vector.reciprocal(recip[:], red[:, 0:1])
            prox_n = wp.tile([K, C], f32r, tag="proxn")
            nc.vector.tensor_scalar_mul(prox_n[:], in0=red[:, 1:], scalar1=recip[:])

            # proxy -> [c, k] layout (identity matmul transposes)
            proxT = wp.tile([128, 4, KP], f32r, tag="proxT")
            for a in range(4):
                ps_t = ppT.tile([128, 128], f32, tag="tr", name="ps_tr2")
                nc.tensor.matmul(ps_t[:, :KP], prox_n[:, a * 128:(a + 1) * 128],
                                 ident[:K, :KP], start=True, stop=True)
                nc.vector.tensor_copy(proxT[:, a, :], ps_t[:, :KP])

            # kk = cbr(cbr(proxy, o1), o2); val = cbr(proxy, d)
            def small_conv(wt, bt, rhs_tile, kin, kout, tag):
                res = wp.tile([128, kout, KP], f32r, tag=tag)
                for o in range(kout):
                    ps = ppM.tile([128, 512], f32, tag="mm", name="ps_sc")
                    ps = ps[:, :KP]
                    for k in range(kin):
                        nc.tensor.matmul(ps[:], wt[:, k, o, :], rhs_tile[:, k, :],
                                         start=(k == 0), stop=(k == kin - 1))
                    nc.scalar.activation(res[:, o, :], ps[:], AF.Relu,
                                         bias=bt[:, o:o + 1], scale=1.0)
                return res

            kk1 = small_conv(wo1, bo1, proxT, 4, 2, "kk1")
            kk = small_conv(wo2, bo2, kk1, 2, 2, "kk")
            val_cb = small_conv(wd, bd, proxT, 4, 2, "valcb")
            valT = wp.tile([K, 2, 128], f32r, tag="valT")
            for o in range(2):
                ps_t = ppT.tile([128, 128], f32, tag="tr", name="ps_tr2")
                nc.tensor.matmul(ps_t[:K, :], val_cb[:, o, 0:K], ident[:],
                                 start=True, stop=True)
                nc.vector.tensor_copy(valT[:, o, :], ps_t[:K, :])

            # ============ B2: attention + f_up + final conv ============
            # Software-pipelined: tile t's attention chain (PE-light, full of
            # ACT/DVE latency) is interleaved with tile t-1's f_up/final conv
            # (PE-heavy) so the in-order PE stream never idles on the chain.
            st = [dict() for _ in range(NT)]

            def att1(t):
                d = st[t]
                ft = bp.tile([128, 4, 512], f32r, tag="F2", bufs=3, name="ft2")
                nc.sync.dma_start(
                    ft[:],
                    feats_d[:, t * 512:(t + 1) * 512].rearrange(
                        "(a p) s -> p a s", p=128).bitcast(f32r))
                q2r = bp.tile([128, 2, 512], f32r, tag="q2r", bufs=3, name="q2r")
                nc.sync.dma_start(q2r[:],
                                  q2_d[t].rearrange("o p s -> p o s").bitcast(f32r))
                ps_log = ppM.tile([128, 512], f32, tag="mm", name="ps_log")
                for k in range(2):
                    nc.tensor.matmul(ps_log[:K, :], kk[:, k, 0:K], q2r[:, k, :],
                                     start=(k == 0), stop=(k == 1))
                e_att = bp.tile([K, 512], f32r, tag="eatt", name="e_att")
                nc.scalar.activation(e_att[:], ps_log[:K, :], AF.Exp, scale=SCALE)
                d["ft"], d["e_att"] = ft, e_att

            def att2a(t):
                d = st[t]
                ps_dn = ppM.tile([128, 512], f32, tag="mm", name="ps_dn")
                nc.tensor.matmul(ps_dn[:1, :], ones[:K, 0:1], d["e_att"][:],
                                 start=True, stop=True)
                rc32 = bp.tile([1, 512], f32, tag="rc32", name="rc32")
                nc.vector.reciprocal_approx_fast(rc32[:], ps_dn[:1, :])
                rc = bp.tile([1, 512], f32r, tag="rc", name="rc")
                nc.vector.tensor_copy(rc[:], rc32[:])
                d["rc"] = rc

            def att2b(t):
                d = st[t]
                ps_bc = ppM.tile([128, 512], f32, tag="mm", name="ps_bc")
                nc.tensor.matmul(ps_bc[:K, :], ones[0:1, 0:K], d["rc"][:],
                                 start=True, stop=True)
                sim = bp.tile([K, 512], f32r, tag="sim", name="sim")
                nc.vector.tensor_mul(sim[:], d["e_att"][:].bitcast(f32), ps_bc[:K, :])
                d["sim"] = sim

            def att3(t):
                d = st[t]
                ctx = bp.tile([128, 2, 512], f32r, tag="ctx", name="ctx")
                for o in range(2):
                    ps = ppM.tile([128, 512], f32, tag="mm")
                    nc.tensor.matmul(ps[:], valT[:, o, :], d["sim"][:],
                                     start=True, stop=True)
                    nc.vector.tensor_copy(ctx[:, o, :], ps[:])
                d["ctx"] = ctx

            def fup(t, orange):
                d = st[t]
                if "ctx2" not in d:
                    d["ctx2"] = bp.tile([128, 4, 512], f32r, tag="ctx2", name="ctx2")
                for o in orange:
                    ps = ppM.tile([128, 512], f32, tag="mm")
                    for k in range(2):
                        nc.tensor.matmul(ps[:], wu[:, k, o, :], d["ctx"][:, k, :],
                                         start=(k == 0), stop=(k == 1))
                    nc.scalar.activation(d["ctx2"][:, o, :], ps[:], AF.Relu,
                                         bias=bu[:, o:o + 1], scale=1.0)

            def final(t, orange):
                d = st[t]
                if "ot" not in d:
                    d["ot"] = bp.tile([128, 4, 512], f32, tag="out", bufs=3,
                                      name="ot")
                ot = d["ot"]
                korder = [4, 5, 6, 7, 0, 1, 2, 3]
                for o in orange:
                    ps = ppM.tile([128, 512], f32, tag="mm")
                    for i, k in enumerate(korder):
                        rhs = d["ctx2"][:, k, :] if k < 4 else d["ft"][:, k - 4, :]
                        nc.tensor.matmul(ps[:], wf[:, k, o, :], rhs,
                                         start=(i == 0), stop=(i == 7))
                    nc.scalar.activation(ot[:, o, :], ps[:], AF.Relu,
                                         bias=bf[:, o:o + 1], scale=1.0)
                if orange[-1] == 3:
                    nc.sync.dma_start(
                        out_d[:, t * 512:(t + 1) * 512].rearrange(
                            "(o p) s -> p o s", p=128),
                        ot[:])
                    st[t] = None

            for t in range(NT + 1):
                if t < NT:
                    att1(t)
                if t >= 1:
                    fup(t - 1, (0, 1))
                if t < NT:
                    att2a(t)
                if t >= 1:
                    fup(t - 1, (2, 3))
                if t < NT:
                    att2b(t)
                if t >= 1:
                    final(t - 1, (0, 1))
                if t < NT:
                    att3(t)
                if t >= 1:
                    final(t - 1, (2, 3))

    nc.compile()
    return nc


def _fold(w, b, s, t):
    """conv+BN fold: y = s*(Wx+b)+t = (s.W)x + (s*b+t)."""
    w = np.asarray(w, np.float32)
    b = np.asarray(b, np.float32)
    s = np.asarray(s, np.float32)
    t = np.asarray(t, np.float32)
    return (s[:, None] * w), (s * b + t)


def kernel(feats, probs,
           wp1, bp1, sp1, tp1, wp2, bp2, sp2, tp2,
           wo1, bo1, so1, to1, wo2, bo2, so2, to2,
           wd, bd, sd, td, wu, bu, su, tu,
           wf, bf, sf, tf, clip_num, _trace=False):
    feats = np.ascontiguousarray(np.asarray(feats, np.float32))
    probs = np.ascontiguousarray(np.asarray(probs, np.float32))

    W1, B1 = _fold(wp1, bp1, sp1, tp1)
    W2, B2 = _fold(wp2, bp2, sp2, tp2)
    WO1, BO1 = _fold(wo1, bo1, so1, to1)
    WO2, BO2 = _fold(wo2, bo2, so2, to2)
    WD, BD = _fold(wd, bd, sd, td)
    WU, BU = _fold(wu, bu, su, tu)
    WF, BF = _fold(wf, bf, sf, tf)

    shared = {
        "wp1T": np.ascontiguousarray(W1.T), "bp1": B1,
        "wp2T": np.ascontiguousarray(W2.T), "bp2": B2,
        "wo1T": np.ascontiguousarray(WO1.T), "bo1": BO1,
        "wo2T": np.ascontiguousarray(WO2.T), "bo2": BO2,
        "wdT": np.ascontiguousarray(WD.T), "bd": BD,
        "wuT": np.ascontiguousarray(WU.T), "bu": BU,
        "wfT": np.ascontiguousarray(WF.T), "bf": BF,
        "ident": np.eye(128, dtype=np.float32),
        "ones": np.ones((128, 32), np.float32),
    }

    fr = feats.reshape(N, C, HW)
    pr = probs.reshape(N, K, HW)
    in_maps = []
    for j in range(8):
        n, h = j // 2, j % 2
        sl = slice(h * HALF, (h + 1) * HALF)
        in_maps.append({
            "feats_half": np.ascontiguousarray(fr[n, :, sl]),
            "probs_half": np.ascontiguousarray(pr[n, :, sl]),
            **shared,
        })

    if "nc" not in _CACHED:
        _CACHED["nc"] = _build_nc()
    nc = _CACHED["nc"]

    res = run_bass_kernel_spmd(nc, in_maps, list(range(8)), trace=_trace)
    out = np.empty((N, OUT, HW), np.float32)
    for j in range(8):
        n, h = j // 2, j % 2
        out[n, :, h * HALF:(h + 1) * HALF] = res.results[j]["out_half"]
    if _trace:
        kernel.last_exec_time_ns = res.exec_time_ns
        kernel.last_results = res
    return out.reshape(N, OUT, H, W)
